# revision 2
# baseline (speedup 1.0000x reference)
"""BiosyntheticCoherenceLoss on 8 Trainium2 NeuronCores — sorted-stream design.

Points are sorted by biosynthetic family (stops last).  All n*(n-1)/2 pairwise
distances are computed exactly once via a strict block-upper-triangle at
128-row granularity, organized as a flat per-core column stream of uniform
128-col matmul slots and 2048-col ACT windows (SPMD: the instruction stream is
identical on every core; per-core differences live entirely in packed tables).

Per column j of the stream: PE computes d2(row, j) - beta via a K=18 bf16
matmul (u = [-2x, 1, |x|^2], w = [y, |y|^2, 1]; pad columns give exactly
-beta -> dist 0); ScalarE applies Sqrt(psum + beta) over each [128, 2048]
window with a free per-row accumulator.  The stream is ordered by accounting
class: GEN (generic off-diag), SAME (pure-strip same-family columns), one
dedicated mixed-boundary window per core (row-extracted), DSAME/DSTOP
diagonal 128x128 blocks, CAL (zero-lhsT -> SqrtACT(beta) calibration).
DVE supplies per-row prefix sums at the (core-uniform) class boundaries that
fall mid-window.

  T = 2*S2_off + D_SAME + D_STOP - n*cal
  M = 2*SAME_total + D_SAME - (#family points)*cal
"""
import numpy as np
import ml_dtypes

import concourse.bass as bass
from concourse import mybir
from concourse.bass_utils import run_bass_kernel_spmd

F32 = mybir.dt.float32
BF16 = mybir.dt.bfloat16
BF = ml_dtypes.bfloat16

N_CORES = 8
D = 16
N = 8192
K = 21
WIN = 2048
SLOT = 128
SLOTS_PER_WIN = WIN // SLOT
PSUM_W = 4096
BETA = 0.25           # Sqrt bias guard (>= |bf16 d2 error|)
NSEG = 4              # input DMA segments

FAM_TABLE = np.array([
    4, 4, 3, 3, 3, 3, 3, 3, 1, 1, 1, 1, 3, 3, 3, 3,
    2, 2, 2, 2, 0, 0, 0, 0, 1, 1, 1, 1, 3, 3, 3, 3,
    4, 4, -1, -1, 5, 5, 0, 0, 1, 1, 1, 1, 1, 1, 0, 0,
    2, 2, -1, 4, 0, 0, 0, 0, 2, 2, 0, 0, 2, 2, 2, 2,
], dtype=np.int64)

PAD = -1   # pad column: dist == 0 vs any real/pad-u row, cal vs pad-u row
CAL = -2   # cal column (only under the zero lhsT)

_PROGRAM_CACHE = {}


# ===========================================================================
# planning
# ===========================================================================
class Chunk:
    """Contiguous stream columns sharing one lhsT."""
    __slots__ = ("lhs", "cols", "cls", "strip")

    def __init__(self, lhs, cols, cls, strip=None):
        self.lhs = lhs          # lhs key: ("s", strip) | ("m", strip, part) | ("z",)
        self.cols = np.asarray(cols, dtype=np.int64)  # sorted-point idx / PAD / CAL
        self.cls = cls
        self.strip = strip


def _build_chunks(fam):
    """fam: [N] family of sorted points (6 == stop). Returns class->list[Chunk],
    plus metadata."""
    cnt = np.bincount(fam, minlength=7)
    bounds = np.cumsum(cnt)
    nstrip = N // 128
    ar = np.arange(N)

    C = {k: [] for k in ("GEN", "SAME", "MIXW", "DSAME", "DSTOP")}
    mini_cal = []          # (cls, lhs, ncal) pad-u x pad-w cal-cell corrections
    mixw_meta = []         # (strip, row_lo_local) rows [row_lo:128] are g-rows

    for s in range(nstrip):
        lo, hi = 128 * s, 128 * s + 128
        f0, f1 = int(fam[lo]), int(fam[hi - 1])
        if f0 == f1:
            cls_d = "DSTOP" if f0 == 6 else "DSAME"
            C[cls_d].append(Chunk(("s", s), ar[lo:hi], cls_d, s))
            if f0 != 6:
                e = int(bounds[f0])
                if e > hi:
                    C["SAME"].append(Chunk(("s", s), ar[hi:e], "SAME", s))
                gen_lo = e
            else:
                gen_lo = hi
            if gen_lo < N:
                C["GEN"].append(Chunk(("s", s), ar[gen_lo:N], "GEN", s))
        else:
            e0 = int(bounds[f0])
            b = e0 - lo                      # f0 rows local [0, b)
            # mini A: f0-rows x f0-cols   (lhs ("m",s,0): rows >= b padded)
            cols = np.full(128, PAD, np.int64)
            cols[:b] = ar[lo:e0]
            clsA = "DSTOP" if f0 == 6 else "DSAME"
            C[clsA].append(Chunk(("m", s, 0), cols, clsA, s))
            mini_cal.append((clsA, (128 - b) * (128 - b)))
            # mini B: g-rows x g-cols     (lhs ("m",s,1): rows < b padded)
            cols = np.full(128, PAD, np.int64)
            cols[:128 - b] = ar[e0:hi]
            clsB = "DSTOP" if f1 == 6 else "DSAME"
            C[clsB].append(Chunk(("m", s, 1), cols, clsB, s))
            mini_cal.append((clsB, b * b))
            # cross: f0-rows x g-cols -> GEN (use lhs A: g-rows padded)
            cols = np.full(128, PAD, np.int64)
            cols[:128 - b] = ar[e0:hi]
            C["GEN"].append(Chunk(("m", s, 0), cols, "GEN", s))
            mini_cal.append(("GEN", (128 - b) * b))
            # g same-cols -> dedicated mixed window
            if f1 != 6:
                e1 = int(bounds[f1])
                if e1 > hi:
                    C["MIXW"].append(Chunk(("s", s), ar[hi:e1], "MIXW", s))
                    mixw_meta.append((s, b))
                gen_lo = e1
            else:
                gen_lo = hi
            if gen_lo < N:
                C["GEN"].append(Chunk(("s", s), ar[gen_lo:N], "GEN", s))

    return C, mini_cal, mixw_meta, cnt, bounds


def _plan(fam):
    """Build per-core streams (lists of (lhs, col)), identical cut layout."""
    C, mini_cal, mixw_meta, cnt, bounds = _build_chunks(fam)

    padc = (("s", 0), PAD)

    # --- GEN + SAME: split chunk lists into 8 contiguous col-count shares ---
    core_streams = [[] for _ in range(N_CORES)]
    cls_ranges = {}
    pos = 0

    for cls in ("GEN", "SAME"):
        flat = []
        for ch in C[cls]:
            cols = ch.cols
            rem = (-len(cols)) % SLOT
            if rem:
                cols = np.concatenate([cols, np.full(rem, PAD, np.int64)])
            flat.append((ch.lhs, cols))
        total = sum(len(c) for _, c in flat)
        per = -(-total // N_CORES)
        per = -(-per // SLOT) * SLOT          # slot-align class share
        # walk chunks, dealing contiguous pieces to cores
        ci, off = 0, 0
        for c in range(N_CORES):
            need = per
            out = core_streams[c]
            while need > 0 and ci < len(flat):
                lhs, cols = flat[ci]
                take = min(need, len(cols) - off)
                out.extend((lhs, int(x)) for x in cols[off:off + take])
                off += take
                need -= take
                if off == len(cols):
                    ci += 1
                    off = 0
            out.extend([padc] * need)
        assert ci == len(flat), "class share distribution bug"
        pos += per
        cls_ranges[cls] = (pos - per, pos)

    # pad to window boundary before MIXW
    gap = (-pos) % WIN
    if gap:
        for c in range(N_CORES):
            core_streams[c].extend([padc] * gap)
        pos += gap
    cls_ranges["PADA"] = (pos - gap, pos)

    # --- MIXW: one full window per core ---
    assert len(C["MIXW"]) <= N_CORES, "more mixed windows than cores"
    for c in range(N_CORES):
        if c < len(C["MIXW"]):
            ch = C["MIXW"][c]
            assert len(ch.cols) <= WIN, "mixed same-range exceeds one window"
            s = [(ch.lhs, int(x)) for x in ch.cols]
            s.extend([(ch.lhs, PAD)] * (WIN - len(s)))
        else:
            s = [padc] * WIN
        core_streams[c].extend(s)
    pos += WIN
    cls_ranges["MIXW"] = (pos - WIN, pos)

    # --- DSAME / DSTOP minis (128-col units) ---
    for cls in ("DSAME", "DSTOP"):
        units = C[cls]
        per_units = -(-len(units) // N_CORES)
        for c in range(N_CORES):
            mine = units[c::N_CORES]
            out = core_streams[c]
            for ch in mine:
                out.extend((ch.lhs, int(x)) for x in ch.cols)
            out.extend([padc] * (SLOT * (per_units - len(mine))))
        pos += SLOT * per_units
        cls_ranges[cls] = (pos - SLOT * per_units, pos)

    # --- CAL tail: fill to >= 512 cal cols and window multiple ---
    tail = (-pos) % WIN
    if tail < 512:
        tail += WIN
    calc = ((("z",), CAL),) * tail
    for c in range(N_CORES):
        core_streams[c].extend(calc)
    pos += tail
    cls_ranges["CAL"] = (pos - tail, pos)

    stream_len = pos
    assert stream_len % WIN == 0
    for c in range(N_CORES):
        assert len(core_streams[c]) == stream_len

    # cuts: class boundaries not on window boundaries (same for all cores)
    cuts = []
    for cls in ("GEN", "SAME", "PADA", "MIXW", "DSAME", "DSTOP"):
        lo, hi = cls_ranges[cls]
        if hi % WIN:
            cuts.append(hi)
    cuts = sorted(set(cuts))

    meta = {
        "cls_ranges": cls_ranges, "cuts": cuts, "stream_len": stream_len,
        "mini_cal": mini_cal, "mixw_meta": mixw_meta, "cnt": cnt,
        "bounds": bounds, "nwin": stream_len // WIN,
        "mixw_count": len(C["MIXW"]),
    }
    return core_streams, meta


# ===========================================================================
# tables
# ===========================================================================
def _tables(core_streams, meta, emb_s, fam):
    """Build per-core packed wt/ut bf16 tables + uniform slot lhs layout."""
    stream_len = meta["stream_len"]
    nslot = stream_len // SLOT

    # u/w per sorted point (fp32 master)
    # exact bf16 self-product: P_i = sum_c bf16(-2x) * bf16(x); the two sq
    # rows on each side carry -P/2 so diagonal d2 cells are ~0 by construction
    m2b = (-2.0 * emb_s).astype(BF).astype(np.float64)
    xb = emb_s.astype(BF).astype(np.float64)
    P = np.einsum("ij,ij->i", m2b, xb)
    h = -0.5 * P
    h_hi = h.astype(BF).astype(np.float64)
    h_lo = (h - h_hi).astype(np.float32)
    u = np.zeros((N, K), np.float32)
    u[:, :16] = -2.0 * emb_s
    u[:, 16] = 1.0
    u[:, 17] = 1.0
    u[:, 18] = h_hi
    u[:, 19] = h_lo
    w = np.zeros((N, K), np.float32)
    w[:, :16] = emb_s
    w[:, 16] = h_hi
    w[:, 17] = h_lo
    w[:, 18] = 1.0
    w[:, 19] = 1.0
    u_pad = np.zeros(K, np.float32)
    u_pad[18] = -BETA
    w_pad = np.zeros(K, np.float32)
    w_pad[16] = -BETA

    ub = u.astype(BF)
    wb = w.astype(BF)
    upad_b = u_pad.astype(BF)
    wpad_b = w_pad.astype(BF)
    zero_b = np.zeros(K, BF)

    def lhs_cols(lhs):
        """[128, K] u-columns for an lhs key."""
        if lhs == ("z",):
            return np.zeros((128, K), BF)
        if lhs[0] == "s":
            s = lhs[1]
            return ub[128 * s:128 * s + 128]
        _, s, part = lhs
        lo = 128 * s
        f0 = fam[lo]
        b = int(np.searchsorted(fam[lo:lo + 128], f0, side="right"))
        out = np.empty((128, K), BF)
        out[:] = upad_b
        if part == 0:
            out[:b] = ub[lo:lo + b]
        else:
            out[b:] = ub[lo + b:lo + 128]
        return out

    in_maps = []
    for c in range(N_CORES):
        st = core_streams[c]
        wt = np.empty((stream_len, K), BF)
        ut = np.empty((nslot * 128, K), BF)
        lhs_cache = {}
        for m in range(nslot):
            lhs = st[m * SLOT][0]
            if lhs not in lhs_cache:
                lhs_cache[lhs] = lhs_cols(lhs)
            ut[m * 128:(m + 1) * 128] = lhs_cache[lhs]
        for j, (lhs, col) in enumerate(st):
            assert lhs == st[(j // SLOT) * SLOT][0], "slot with mixed lhs"
            if col == PAD:
                wt[j] = wpad_b
            elif col == CAL:
                wt[j] = zero_b
            else:
                wt[j] = wb[col]
        in_maps.append({
            "wt": wt.T.copy(),
            "ut": ut.T.copy(),
            "cst": np.full((128, 1), BETA, np.float32),
        })
    return in_maps


# ===========================================================================
# bass program
# ===========================================================================
def _build_program(stream_len, cuts):
    key = (stream_len, tuple(cuts))
    if key in _PROGRAM_CACHE:
        return _PROGRAM_CACHE[key]

    nwin = stream_len // WIN
    nslot = stream_len // SLOT
    ncut = len(cuts)
    nacc = nwin + ncut
    # progressive segment ends (in stream cols): small first for a fast start
    seg_ends = [WIN, 4 * WIN]
    while seg_ends[-1] < stream_len:
        seg_ends.append(min(stream_len, seg_ends[-1] * 2))
    seg_ends[-1] = stream_len
    seg_ends = sorted(set(seg_ends))
    NSEGP = len(seg_ends)
    assert NSEGP <= 6

    # cut j -> (window idx, local prefix width)
    cutw = [(p // WIN, p % WIN) for p in cuts]
    # cuts per window cumulative (after window w complete)
    cut_cum = [0] * (nwin + 1)
    for (wi, _) in cutw:
        cut_cum[wi + 1] += 1
    for i in range(1, nwin + 1):
        cut_cum[i] += cut_cum[i - 1]

    nc = bass.Bass()
    wt = nc.declare_dram_parameter("wt", [K, stream_len], BF16, isOutput=False)
    ut = nc.declare_dram_parameter("ut", [K, nslot * 128], BF16, isOutput=False)
    cst = nc.declare_dram_parameter("cst", [128, 1], F32, isOutput=False)
    acc_out = nc.declare_dram_parameter("acc", [128, nacc], F32, isOutput=True)

    with (
        nc.sbuf_tensor([K, stream_len], BF16) as wt_t,
        nc.sbuf_tensor([K, nslot * 128], BF16) as ut_t,
        nc.sbuf_tensor([128, 1], F32) as eps_t,
        nc.sbuf_tensor([128, nacc], F32) as acc_t,
        nc.sbuf_tensor([128, 2 * WIN], BF16) as scr_t,
        nc.psum_tensor([128, PSUM_W], F32) as ps,
        nc.semaphore() as seg0_sem,
        nc.semaphore() as seg1_sem,
        nc.semaphore() as seg2_sem,
        nc.semaphore() as seg3_sem,
        nc.semaphore() as seg4_sem,
        nc.semaphore() as seg5_sem,
        nc.semaphore() as cst_sem,
        nc.semaphore() as mm_sem,
        nc.semaphore() as act_sem,
        nc.semaphore() as dve_sem,
        nc.Block() as block,
    ):
        seg_sems = [seg0_sem, seg1_sem, seg2_sem, seg3_sem, seg4_sem, seg5_sem]

        @block.sync
        def _(sync):
            sync.dma_start(out=eps_t[:], in_=cst[:]).then_inc(cst_sem, 16)
            lo = 0
            for g, hi in enumerate(seg_ends):
                sync.dma_start(out=wt_t[:, lo:hi],
                               in_=wt[:, lo:hi]).then_inc(seg_sems[g], 16)
                sync.dma_start(out=ut_t[:, lo:hi],
                               in_=ut[:, lo:hi]).then_inc(seg_sems[g], 16)
                lo = hi

        @block.tensor
        def _(tensor):
            seg_have = 0
            for m in range(nslot):
                end = (m + 1) * SLOT
                need = 0
                while seg_ends[need] < end:
                    need += 1
                if need + 1 > seg_have:
                    for g in range(seg_have, need + 1):
                        tensor.wait_ge(seg_sems[g], 32)
                    seg_have = need + 1
                wi = m // (SLOTS_PER_WIN * 2)      # psum round (2 windows)
                w_of_m = m // SLOTS_PER_WIN
                if w_of_m >= 2 and m % SLOTS_PER_WIN == 0:
                    tensor.wait_ge(act_sem, w_of_m - 1)
                p0 = (m * SLOT) % PSUM_W
                mm = nc.tensor.matmul(
                    ps[:, p0:p0 + SLOT],
                    ut_t[:, m * 128:(m + 1) * 128],
                    wt_t[:, m * SLOT:(m + 1) * SLOT],
                    start=True, stop=True)
                if m % SLOTS_PER_WIN == SLOTS_PER_WIN - 1:
                    mm.then_inc(mm_sem, 1)

        @block.scalar
        def _(scalar):
            scalar.wait_ge(cst_sem, 16)
            for wi in range(nwin):
                scalar.wait_ge(mm_sem, wi + 1)
                if wi >= 2 and cut_cum[wi - 1] > 0 and cut_cum[wi - 1] > cut_cum[wi - 2]:
                    scalar.wait_ge(dve_sem, cut_cum[wi - 1])
                r = (wi % 2) * WIN
                nc.scalar.activation(
                    scr_t[:, r:r + WIN],
                    ps[:, r:r + WIN],
                    mybir.ActivationFunctionType.Sqrt,
                    bias=eps_t.ap(),
                    accum_out=acc_t[:, wi:wi + 1],
                ).then_inc(act_sem, 1)
            if ncut:
                scalar.wait_ge(dve_sem, ncut)
            nc.scalar.dma_start(out=acc_out[:], in_=acc_t[:]).then_inc(cst_sem, 16)

        @block.vector
        def _(vector):
            for j, (wi, wloc) in enumerate(cutw):
                vector.wait_ge(act_sem, wi + 1)
                r = (wi % 2) * WIN
                nc.vector.tensor_reduce(
                    acc_t[:, nwin + j:nwin + j + 1],
                    scr_t[:, r:r + wloc],
                    axis=mybir.AxisListType.X,
                    op=mybir.AluOpType.add).then_inc(dve_sem, 1)

    prog = nc
    _PROGRAM_CACHE[key] = prog
    return prog


# ===========================================================================
# host assembly
# ===========================================================================
def _assemble(results, meta, core_streams, fam):
    nwin = meta["nwin"]
    cuts = meta["cuts"]
    cls_ranges = meta["cls_ranges"]
    cnt = meta["cnt"].astype(np.float64)

    # per-core per-class scalar sums from window accums +/- cut prefixes
    # build map: for each class, list of (win, frac) pieces:
    #   full windows inside class -> whole accum
    #   boundary windows -> prefix/suffix via cut values
    cut_pos = {p: j for j, p in enumerate(cuts)}

    def class_sum(acc, lo, hi):
        """acc: [128, nacc] fp64; total over stream cols [lo, hi) (all rows)."""
        tot = 0.0
        w0, w1 = lo // WIN, (hi + WIN - 1) // WIN
        for wi in range(w0, w1):
            wlo, whi = wi * WIN, wi * WIN + WIN
            a = acc[:, wi].sum()
            if lo > wlo:
                assert lo in cut_pos, (lo, cuts)
                a -= acc[:, nwin + cut_pos[lo]].sum()
            if hi < whi:
                assert hi in cut_pos, (hi, cuts)
                a -= (acc[:, wi].sum() - acc[:, nwin + cut_pos[hi]].sum())
            tot += a
        return tot

    S2_off = 0.0
    SAME_tot = 0.0
    D_same = 0.0
    D_stop = 0.0
    cal_sum = 0.0
    cal_cnt = 0

    mixw_lo, mixw_hi = cls_ranges["MIXW"]
    assert mixw_lo % WIN == 0 and mixw_hi % WIN == 0
    mixw_win = mixw_lo // WIN

    for c, res in enumerate(results):
        acc = res["acc"].astype(np.float64)
        for cls in ("GEN", "SAME", "PADA"):
            lo, hi = cls_ranges[cls]
            if hi > lo:
                v = class_sum(acc, lo, hi)
                S2_off += v
                if cls == "SAME":
                    SAME_tot += v
        # mixed window
        S2_off += acc[:, mixw_win].sum()
        if c < meta["mixw_count"]:
            s, b = meta["mixw_meta"][c]
            SAME_tot += acc[b:, mixw_win].sum()
        for cls, tgt in (("DSAME", "ds"), ("DSTOP", "dt")):
            lo, hi = cls_ranges[cls]
            if hi > lo:
                v = class_sum(acc, lo, hi)
                if tgt == "ds":
                    D_same += v
                else:
                    D_stop += v
        lo, hi = cls_ranges["CAL"]
        cal_sum += class_sum(acc, lo, hi)
        cal_cnt += 128 * (hi - lo)

    cal = cal_sum / cal_cnt

    # mini pad-u x pad-w corrections (each such cell contributed `cal`)
    for cls, ncal in meta["mini_cal"]:
        corr = ncal * cal
        if cls == "DSAME":
            D_same -= corr
        elif cls == "DSTOP":
            D_stop -= corr
        else:
            S2_off -= corr

    n = float(N)
    nfam = float(cnt[:6].sum())
    T = 2.0 * S2_off + D_same + D_stop - n * cal
    M = 2.0 * SAME_tot + D_same - nfam * cal
    return T, M, cal


def _finish(T, M, cnt):
    cnt = cnt.astype(np.float64)
    same_count = float((cnt[:6] ** 2).sum())
    total_count = float(N) * N
    eps = 1e-10
    same_d = M / (same_count + eps)
    diff_d = (T - M) / ((total_count - same_count) + eps)
    loss = same_d - 0.5 * diff_d + 1.0
    return np.float32(max(loss, 0.0))


# ===========================================================================
# driver
# ===========================================================================
def _prepare(codon_embeddings, codon_indices):
    emb = np.ascontiguousarray(codon_embeddings, dtype=np.float32).reshape(-1, D)
    idx = np.asarray(codon_indices).reshape(-1).astype(np.int64)
    assert emb.shape[0] == N

    famr = FAM_TABLE[idx]
    fam = np.where(famr < 0, 6, famr)
    order = np.argsort(fam, kind="stable")
    emb_s = emb[order]
    fam_s = fam[order]
    core_streams, meta = _plan(fam_s)
    in_maps = _tables(core_streams, meta, emb_s, fam_s)
    return in_maps, core_streams, meta, fam_s


def simulate(codon_embeddings, codon_indices):
    """Pure-numpy emulation of the device accums -> loss (algebra check)."""
    emb = np.ascontiguousarray(codon_embeddings, dtype=np.float32).reshape(-1, D)
    idx = np.asarray(codon_indices).reshape(-1).astype(np.int64)
    famr = FAM_TABLE[idx]
    fam = np.where(famr < 0, 6, famr)
    order = np.argsort(fam, kind="stable")
    emb_s = emb[order].astype(np.float64)
    fam_s = fam[order]
    sq_s = (emb_s ** 2).sum(1)

    core_streams, meta = _plan(fam_s)
    nwin = meta["nwin"]
    cuts = meta["cuts"]
    cal_v = np.sqrt(BETA)

    m2b = (-2.0 * emb_s).astype(BF).astype(np.float64)
    xb = emb_s.astype(BF).astype(np.float64)
    P = np.einsum("ij,ij->i", m2b, xb)
    h = -0.5 * P
    h_hi = h.astype(BF).astype(np.float64)
    h_lo = h - h_hi
    u = np.zeros((N, K))
    u[:, :16] = -2.0 * emb_s
    u[:, 16] = 1.0
    u[:, 17] = 1.0
    u[:, 18] = h_hi
    u[:, 19] = h_lo
    w = np.zeros((N, K))
    w[:, :16] = emb_s
    w[:, 16] = h_hi
    w[:, 17] = h_lo
    w[:, 18] = 1.0
    w[:, 19] = 1.0
    u = u.astype(BF).astype(np.float64)
    w = w.astype(BF).astype(np.float64)
    u_pad = np.zeros(K); u_pad[18] = -BETA
    w_pad = np.zeros(K); w_pad[16] = -BETA

    def lhs_mat(lhs):
        if lhs == ("z",):
            return np.zeros((128, K))
        if lhs[0] == "s":
            s = lhs[1]
            return u[128 * s:128 * s + 128]
        _, s, part = lhs
        lo = 128 * s
        f0 = fam_s[lo]
        b = int(np.searchsorted(fam_s[lo:lo + 128], f0, side="right"))
        out = np.tile(u_pad, (128, 1))
        if part == 0:
            out[:b] = u[lo:lo + b]
        else:
            out[b:] = u[lo + b:lo + 128]
        return out

    results = []
    for c in range(N_CORES):
        st = core_streams[c]
        SL = len(st)
        wmat = np.empty((SL, K))
        lmat = np.empty((SL, 128, ))
        # build column values: d2col[row, j] then dist
        dist = np.empty((128, SL))
        j0 = 0
        # process per slot for lhs efficiency
        for m in range(SL // SLOT):
            lhs = st[m * SLOT][0]
            L = lhs_mat(lhs)               # [128, K]
            Wc = np.empty((SLOT, K))
            for t in range(SLOT):
                _, col = st[m * SLOT + t]
                if col == PAD:
                    Wc[t] = w_pad
                elif col == CAL:
                    Wc[t] = 0.0
                else:
                    Wc[t] = w[col]
            d2 = L @ Wc.T                  # [128, SLOT]
            dist[:, m * SLOT:(m + 1) * SLOT] = np.sqrt(np.maximum(d2 + BETA, 0))
        acc = np.zeros((128, nwin + len(cuts)))
        for wi in range(nwin):
            acc[:, wi] = dist[:, wi * WIN:(wi + 1) * WIN].sum(1)
        for j, p in enumerate(cuts):
            wi, wloc = p // WIN, p % WIN
            acc[:, nwin + j] = dist[:, wi * WIN:wi * WIN + wloc].sum(1)
        results.append({"acc": acc})

    T, M, cal = _assemble(results, meta, core_streams, fam_s)
    # correct for exact-sim cal (sqrt(BETA)) vs measured: identical here
    loss = _finish(T, M, meta["cnt"])
    return loss, T, M, cal


def _run(codon_embeddings, codon_indices, trace=False):
    in_maps, core_streams, meta, fam_s = _prepare(codon_embeddings, codon_indices)
    nc = _build_program(meta["stream_len"], meta["cuts"])
    last_exc = None
    vals = []
    r = None
    for attempt in range(6):
        try:
            ri = run_bass_kernel_spmd(nc, in_maps, list(range(N_CORES)), trace=trace)
        except Exception as e:
            last_exc = e
            continue
        if not all(np.isfinite(res["acc"]).all() for res in ri.results):
            continue
        T, M, cal = _assemble(ri.results, meta, core_streams, fam_s)
        v = float(_finish(T, M, meta["cnt"]))
        vals.append(v)
        r = ri
        if any(abs(v - u) <= 1e-5 * max(abs(v), 1.0) for u in vals[:-1]):
            break
        if trace and len(vals) >= 1:
            break
    if r is None:
        raise last_exc
    T, M, cal = _assemble(r.results, meta, core_streams, fam_s)
    return _finish(T, M, meta["cnt"]), r


def kernel(codon_embeddings, codon_indices) -> np.ndarray:
    out, _ = _run(codon_embeddings, codon_indices, trace=False)
    return np.asarray(out, dtype=np.float32)


if __name__ == "__main__":
    import reference
    inputs = {k: np.asarray(v) for k, v in reference.setup_inputs().items()}
    emb = np.asarray(inputs["codon_embeddings"], np.float64).reshape(-1, 16)
    idxs = np.asarray(inputs["codon_indices"]).reshape(-1)
    famr = FAM_TABLE[idxs]
    sq = (emb ** 2).sum(1)
    d2 = sq[:, None] + sq[None, :] - 2.0 * emb @ emb.T
    np.fill_diagonal(d2, 0.0)
    dist = np.sqrt(np.maximum(d2, 0))
    same = (famr[:, None] == famr[None, :]) & (famr[:, None] >= 0)
    Texp = dist.sum()
    Mexp = dist[same].sum()
    loss, T, M, cal = simulate(inputs["codon_embeddings"], inputs["codon_indices"])
    print(f"sim T={T:.2f} exp {Texp:.2f} rel {(T-Texp)/Texp:.2e}")
    print(f"sim M={M:.2f} exp {Mexp:.2f} rel {(M-Mexp)/Mexp:.2e}")
    same_sum = same.sum()
    sd = Mexp / (same_sum + 1e-10)
    dd = (Texp - Mexp) / (8192.0 * 8192 - same_sum + 1e-10)
    lexp = max(sd - 0.5 * dd + 1.0, 0.0)
    print(f"sim loss={loss} expect {lexp}  rel {(float(loss)-lexp)/lexp:.2e}")
